# revision 12
# baseline (speedup 1.0000x reference)
"""Multi-head attention with random-synthesizer blend + mask, on 8 Trainium2
NeuronCores.

Sharding: data-parallel over batch (B=8 -> one batch element per core).

Per-core algorithm (S=1024, D=1024, H=16, HD=64), all layouts [partition, free]:
  - q_T = c1*(Wq^T x^T + bq) in [d_out, s] layout; k_T likewise (scale 1);
    v in natural [s, d_out] layout interleaved with all-ones 64-column blocks
    (used to compute softmax sums for free in the AV matmul).
  - x^T and mask^T tiles come straight from HBM via hardware xbar transpose
    reads (inputs are pre-cast to fp16 on the host, making the 2-byte
    transpose path legal with no DRAM bounce). mask^T is converted in-place
    to lmask = (mask-1)*30000 so the mask folds ADDITIVELY into the
    synthesizer scores: masked positions exp() to exactly 0.
  - Attention runs per head-PAIR so the two K=64 score matmuls of the pair
    land in different PE row-groups (partitions 0:64 vs 64:128) and execute
    concurrently. Per (pair, k-chunk): synm = synT_scaled + lmaskT (DVE),
    scores psum[128,1024] = kT^T qT += identity @ synm, p = exp(psum) in one
    [128,1024] ACT call, out/sums accumulated = [v|ones]^T p. The AV matmuls
    are emitted one k-chunk behind the score matmuls so the PE never waits
    on the exp (software pipelining; the PE queue is FIFO).
  - Normalization is pipelined: each pair's bundle is split into three
    stages (evac+swap / reciprocal / multiply) emitted one k-chunk apart
    inside the NEXT pair's loop, so every stage's dependencies are complete
    before the engine FIFOs reach it (no head-of-line blocking). The
    reciprocal runs as an elementwise divide on the otherwise-idle GPSIMD
    engine (the DVE reciprocal costs 6.5us per pair and would make DVE the
    critical engine; ACT Ln/Exp would thrash activation table sets).
  - A short burst of dummy matmuls at the start warms the PE HAM clock gate
    (cold PE runs at 1.2 GHz vs 2.4 GHz warm) while the first DMAs land.

fp16 is used for matmul operands (fp32 matmuls stream 4x slower); PSUM
accumulation stays fp32.

Host-side prep is limited to dtype casts/layout of parameters and inputs
(fp16 casts of x/mask/weights, transpose+(1-alpha)-scale of the synthesizer
scores), the sigmoid of the scalar alpha parameter, and folding the
zero-cost bias identity bo' = bv @ Wo + bo (exact: softmax weights sum to
1). alpha is folded into compiled constants; the program is rebuilt if
alpha changes.
"""

import math
import sys

sys.path.insert(0, "/opt/trn_rl_repo")

import numpy as np

import concourse.tile as tile
import concourse.mybir as mybir
from concourse import bacc
from concourse.bass_utils import run_bass_kernel_spmd
from concourse.masks import make_identity

B, S, D, H = 8, 1024, 1024, 16
HD = D // H  # 64
N_CORES = 8
P = 128
SC = S // P  # 8
DC = D // P  # 8
NQ = 512
QC = S // NQ  # 2

f32 = mybir.dt.float32
fp16 = mybir.dt.float16
i32 = mybir.dt.int32
AF = mybir.ActivationFunctionType
OP = mybir.AluOpType

# knobs
RECIP_ENGINE = "dve"  # gpsimd divide is rejected by walrus
SYNM_VIA_ACCUM_DMA = True  # build synm = lmask + syn via SWDGE accumulate-DMA
WARMUP_MMS = 12

# test harness knobs (the grading entry point `kernel` leaves these alone)
TRACE = False
TRACE_TMPDIR = None
LAST_RESULTS = None

_CACHE = {}


def _emit(nc, tc, dram, c1):
    xin = {"q": dram["xq"], "k": dram["xk"], "v": dram["xv"]}
    w_d = {"q": dram["wq"], "k": dram["wk"], "v": dram["wv"], "o": dram["wo"]}
    msk_d, syn_d, out_d = dram["msk"], dram["syn"], dram["out"]

    with tc.tile_pool(name="pers", bufs=1) as pers:
        # ---- constants ---------------------------------------------------
        ident = pers.tile([P, P], fp16, tag="ident")
        make_identity(nc, ident[:])
        ones_h = pers.tile([1, P], fp16, tag="ones_h")
        nc.vector.memset(ones_h[:], 1.0)
        ones_v = pers.tile([P, S], fp16, tag="ones_v")
        nc.vector.memset(ones_v[:], 1.0)

        # bo' = bv @ Wo + bo, prepared by the host into dram["boeff"]
        bo_sb = pers.tile([1, D], fp16, tag="bo_sb")

        # ---- persistent activations --------------------------------------
        qT = [pers.tile([P, S], fp16, tag=f"qT{i}", name=f"qT{i}") for i in range(DC)]
        kT = [pers.tile([P, S], fp16, tag=f"kT{i}", name=f"kT{i}") for i in range(DC)]
        # lmaskT[kc] = (maskT - 1) * 30000  (0 where visible, -30000 masked)
        lmaskT = [pers.tile([P, S], fp16, tag=f"lm{i}", name=f"lm{i}")
                  for i in range(SC)]

        def load_w_chunks(nm, wpool, wbufs=2):
            tiles = []
            for ci in range(DC):
                t = wpool.tile([P, D], fp16, tag=f"w{ci}", bufs=wbufs, name=f"w{nm}{ci}")
                nc.sync.dma_start(out=t[:], in_=w_d[nm][ci * P:(ci + 1) * P, :])
                tiles.append(t)
            return tiles

        # ================= phase 1: projections ==========================
        with (
            tc.tile_pool(name="prolog", bufs=1) as pro,
            tc.tile_pool(name="ps1", bufs=1, space="PSUM") as ps1,
        ):
            # PE warmup: dependency-free matmuls so HAM un-throttles
            # (1.2 -> 2.4 GHz) while the first DMAs land.
            wscr = pro.tile([P, NQ], fp16, tag="wscr")
            nc.vector.memset(wscr[:], 0.25)
            wup = ps1.tile([P, S], f32, tag="mmp", bufs=4, name="wup")
            for _ in range(WARMUP_MMS):
                nc.tensor.matmul(wup[:, 0:NQ], ident[:], wscr[:],
                                 start=True, stop=True)

            def transpose_in(x_d, dst_tiles):
                for di in range(DC):
                    nc.sync.dma_start_transpose(
                        out=dst_tiles[di][:], in_=x_d[:, di * P:(di + 1) * P]
                    )

            # q_T / k_T: [d_out, s]; di-outer so the stationary operand
            # (weight chunk) is shared by the sq pair
            bqk_sb = {}
            for nm, dst, scale in (("q", qT, c1), ("k", kT, 1.0)):
                xT = [pro.tile([P, S], fp16, tag=f"xT{i}", bufs=2, name=f"xT{nm}{i}")
                      for i in range(DC)]
                wt = []
                for di in range(DC):
                    nc.sync.dma_start_transpose(
                        out=xT[di][:], in_=xin[nm][:, di * P:(di + 1) * P])
                    t = pro.tile([P, D], fp16, tag=f"w{di}", bufs=2,
                                 name=f"w{nm}{di}")
                    nc.sync.dma_start(out=t[:], in_=w_d[nm][di * P:(di + 1) * P, :])
                    wt.append(t)
                t = pers.tile([P, DC], f32, tag=f"b{nm}", name=f"b{nm}")
                nc.sync.dma_start(
                    out=t[:], in_=dram["b" + nm].rearrange("(c p) -> p c", p=P))
                bqk_sb[nm] = t
                if nm == "q" and c1 != 1.0:
                    nc.vector.tensor_scalar(
                        out=t[:], in0=t[:], scalar1=float(c1),
                        scalar2=None, op0=OP.mult,
                    )
                for do in range(DC):
                    ps = ps1.tile([P, S], f32, tag="mmp", bufs=4, name="psp")
                    for di in range(DC):
                        for sq in range(QC):
                            nc.tensor.matmul(
                                ps[:, sq * NQ:(sq + 1) * NQ],
                                wt[di][:, do * P:(do + 1) * P],
                                xT[di][:, sq * NQ:(sq + 1) * NQ],
                                start=(di == 0),
                                stop=(di == DC - 1),
                            )
                    nc.scalar.activation(
                        out=dst[do][:], in_=ps[:],
                        func=AF.Identity, bias=bqk_sb[nm][:, do:do + 1],
                        scale=float(scale),
                    )

            b0 = pro.tile([1, D], f32, tag="braw")
            nc.sync.dma_start(out=b0[:], in_=dram["boeff"][None, :])
            nc.vector.tensor_copy(out=bo_sb[:], in_=b0[:])

            # v natural [s, d_out] into interleaved [v|ones] blocks (fp16);
            # dq-inner so the stationary operand (xT chunk) is shared
            v_sb = [pers.tile([P, H * P], fp16, tag=f"v{i}", name=f"v{i}")
                    for i in range(SC)]
            xT = [pro.tile([P, S], fp16, tag=f"xT{i}", bufs=2, name=f"xTv{i}")
                  for i in range(DC)]
            transpose_in(xin["v"], xT)
            wt = load_w_chunks("v", pro)
            for sc in range(SC):
                nc.vector.memset(v_sb[sc][:], 1.0)
            for sc in range(SC):
                ps = ps1.tile([P, S], f32, tag="mmp", bufs=4, name="psv")
                for di in range(DC):
                    for dq in range(QC):
                        nc.tensor.matmul(
                            ps[:, dq * NQ:(dq + 1) * NQ],
                            xT[di][:, sc * P:(sc + 1) * P],
                            wt[di][:, dq * NQ:(dq + 1) * NQ],
                            start=(di == 0),
                            stop=(di == DC - 1),
                        )
                # heads h = j here; v block of h at col h*128 + 64*(h&1).
                # Two strided copies instead of 16 tiny ones.
                pv = ps[:].rearrange("p (b c) -> p b c", c=2 * HD)
                dv = v_sb[sc][:].rearrange("p (b c) -> p b c", c=2 * P)
                nc.scalar.copy(out=dv[:, :, 0:HD], in_=pv[:, :, 0:HD])
                nc.scalar.copy(out=dv[:, :, 3 * HD:4 * HD],
                               in_=pv[:, :, HD:2 * HD])

            # mask^T + lmask transform (needed only at attention time)
            for kb in range(SC):
                nc.sync.dma_start_transpose(
                    out=lmaskT[kb][:], in_=msk_d[:, kb * P:(kb + 1) * P]
                )
                nc.vector.tensor_scalar(
                    out=lmaskT[kb][:], in0=lmaskT[kb][:],
                    scalar1=1.0, scalar2=30000.0,
                    op0=OP.subtract, op1=OP.mult,
                )


        # ================= phase 2: attention ============================
        otnp_cm = tc.tile_pool(name="otnp", bufs=1)
        otnp = otnp_cm.__enter__()
        wop_cm = tc.tile_pool(name="wo", bufs=1)
        wop = wop_cm.__enter__()
        otn = [otnp.tile([P, S], fp16, tag=f"otn{i}", name=f"otn{i}")
               for i in range(DC)]
        otn_raw = [otnp.tile([P, S], fp16, tag=f"otr{i}", name=f"otr{i}")
                   for i in range(DC)]
        sums_sb = [otnp.tile([P, S], fp16, tag=f"sus{i}", name=f"sus{i}")
                   for i in range(DC)]
        # prefetch Wo (DMA has slack during attention)
        wt_o = load_w_chunks("o", wop, wbufs=1)
        with (
            tc.tile_pool(name="attn", bufs=1) as ap,
            tc.tile_pool(name="ps2", bufs=1, space="PSUM") as ps2,
        ):
            def make_norm_stages(hp, pav_box):
                h0, h1 = 2 * hp, 2 * hp + 1

                def s1_evac_swap():
                    pav = pav_box["pav"]
                    for i, hh in enumerate((h0, h1)):
                        olo, slo = HD * (hh % 2), HD * (1 - hh % 2)
                        nc.vector.tensor_copy(
                            out=otn_raw[hp][olo:olo + HD, :],
                            in_=pav[i][olo:olo + HD, :],
                        )
                        nc.vector.tensor_copy(
                            out=sums_sb[hp][slo:slo + HD, :],
                            in_=pav[i][slo:slo + HD, :],
                        )
                    rt = ap.tile([P, S], fp16, tag="rtm", bufs=2, name=f"rt{hp}")
                    nc.sync.dma_start(out=rt[0:HD, :], in_=sums_sb[hp][HD:P, :])
                    nc.sync.dma_start(out=rt[HD:P, :], in_=sums_sb[hp][0:HD, :])
                    st["rt"] = rt

                def s2_recip():
                    rec = ap.tile([P, S], fp16, tag="rec", bufs=2, name=f"rc{hp}")
                    with nc.allow_low_precision(reason="softmax denominators; fp16 rel err 5e-4 is fine"):
                        nc.vector.reciprocal(out=rec[:], in_=st["rt"][:])
                    st["rec"] = rec

                def s3_mult():
                    nc.vector.tensor_tensor(
                        out=otn[hp][:], in0=otn_raw[hp][:], in1=st["rec"][:],
                        op=OP.mult,
                    )

                st = {}
                return [None, s1_evac_swap, s2_recip, s3_mult]

            def make_av_emitter(hp, pav_box, p0, p1, kc):
                def emit():
                    if "pav" not in pav_box:
                        pav_box["pav"] = [
                            ps2.tile([P, S], f32, tag=f"av{i}", bufs=1,
                                     name=f"pav{hp}_{i}")
                            for i in range(2)
                        ]
                    pav = pav_box["pav"]
                    for i, p in enumerate((p0, p1)):
                        hh = 2 * hp + i
                        for sq in range(QC):
                            sl = slice(sq * NQ, (sq + 1) * NQ)
                            nc.tensor.matmul(
                                pav[i][:, sl],
                                v_sb[kc][:, hh * P:(hh + 1) * P], p[:, sl],
                                start=(kc == 0), stop=(kc == SC - 1),
                            )
                return emit

            pending = []
            prev_av = None
            for hp in range(H // 2):
                h0, h1 = 2 * hp, 2 * hp + 1
                pav_box = {}
                for kc in range(SC):
                    syns = []
                    for i, hh in enumerate((h0, h1)):
                        synm = ap.tile([P, S], fp16, tag="synm", bufs=6,
                                       name=f"sm{hh}_{kc}")
                        if SYNM_VIA_ACCUM_DMA:
                            # prefill with lmask, then stream syn from HBM
                            # with an inline add (SWDGE accumulate-DMA): the
                            # add costs no DVE time at all
                            nc.vector.tensor_copy(out=synm[:], in_=lmaskT[kc][:])
                            nc.gpsimd.dma_start(
                                out=synm[:],
                                in_=syn_d[hh, kc * P:(kc + 1) * P, :],
                                accum_op=OP.add,
                            )
                        else:
                            syn_t = ap.tile([P, S], fp16, tag="synT", bufs=6,
                                            name=f"sy{hh}_{kc}")
                            nc.sync.dma_start(
                                out=syn_t[:],
                                in_=syn_d[hh, kc * P:(kc + 1) * P, :],
                            )
                            nc.vector.tensor_tensor(
                                out=synm[:], in0=syn_t[:], in1=lmaskT[kc][:],
                                op=OP.add,
                            )
                        syns.append(synm)
                    pss = [ps2.tile([P, S], f32, tag="mm", bufs=2,
                                    name=f"pss{i}") for i in range(2)]
                    # score matmuls, pair-interleaved: h0 uses PE rows 0:64,
                    # h1 rows 64:128 -> adjacent matmuls run concurrently
                    for sq in range(QC):
                        sl = slice(sq * NQ, (sq + 1) * NQ)
                        for i in range(2):
                            nc.tensor.matmul(
                                pss[i][:, sl],
                                kT[hp][i * HD:(i + 1) * HD, kc * P:(kc + 1) * P],
                                qT[hp][i * HD:(i + 1) * HD, sl],
                                start=True, stop=False,
                            )
                    for sq in range(QC):
                        sl = slice(sq * NQ, (sq + 1) * NQ)
                        for i in range(2):
                            nc.tensor.matmul(
                                pss[i][:, sl], ident[:], syns[i][:, sl],
                                start=False, stop=True,
                            )
                    # one stage of the pair-before-last's normalization, then
                    # the AV matmuls lagging one k-chunk behind (the lag runs
                    # across pair boundaries, so the PE stream never waits on
                    # a just-issued exp)
                    if pending:
                        s = pending.pop(0)
                        if s is not None:
                            s()
                    if prev_av is not None:
                        prev_av()
                    ps_ = []
                    for i in range(2):
                        p = ap.tile([P, S], fp16, tag="p", bufs=5, name=f"p{i}")
                        nc.scalar.activation(out=p[:], in_=pss[i][:], func=AF.Exp)
                        ps_.append(p)
                    prev_av = make_av_emitter(hp, pav_box, ps_[0], ps_[1], kc)
                pending = make_norm_stages(hp, pav_box)

            prev_av()
            while pending:
                s = pending.pop(0)
                if s is not None:
                    s()

        # ================= phase 3: output projection ====================
        with tc.tile_pool(name="ps3", bufs=1, space="PSUM") as ps3:
            for sc in range(SC):
                ps = ps3.tile([P, S], f32, tag="mmo", bufs=3, name="pso")
                for ci in range(DC):
                    for dq in range(QC):
                        sl = slice(dq * NQ, (dq + 1) * NQ)
                        nc.tensor.matmul(
                            ps[:, sl],
                            otn[ci][:, sc * P:(sc + 1) * P],
                            wt_o[ci][:, sl],
                            start=(ci == 0), stop=False,
                        )
                for dq in range(QC):
                    sl = slice(dq * NQ, (dq + 1) * NQ)
                    nc.tensor.matmul(
                        ps[:, sl], ones_h[:, :P], bo_sb[:, sl],
                        start=False, stop=True,
                    )
                osb = wop.tile([P, S], f32, tag="osb", bufs=3, name="osb")
                nc.scalar.copy(out=osb[:], in_=ps[:])
                nc.sync.dma_start(
                    out=out_d[sc * P:(sc + 1) * P, :], in_=osb[:],
                )
        wop_cm.__exit__(None, None, None)
        otnp_cm.__exit__(None, None, None)


def _build(c1):
    nc = bacc.Bacc("TRN2", debug=False)
    dram = {
        "xq": nc.declare_dram_parameter("xq", [S, D], fp16, isOutput=False),
        "xk": nc.declare_dram_parameter("xk", [S, D], fp16, isOutput=False),
        "xv": nc.declare_dram_parameter("xv", [S, D], fp16, isOutput=False),
        "msk": nc.declare_dram_parameter("msk", [S, S], fp16, isOutput=False),
        "wq": nc.declare_dram_parameter("wq", [D, D], fp16, isOutput=False),
        "wk": nc.declare_dram_parameter("wk", [D, D], fp16, isOutput=False),
        "wv": nc.declare_dram_parameter("wv", [D, D], fp16, isOutput=False),
        "wo": nc.declare_dram_parameter("wo", [D, D], fp16, isOutput=False),
        "bq": nc.declare_dram_parameter("bq", [D], f32, isOutput=False),
        "bk": nc.declare_dram_parameter("bk", [D], f32, isOutput=False),
        "boeff": nc.declare_dram_parameter("boeff", [D], f32, isOutput=False),
        # pre-transposed [H, S_k, S_q] and pre-scaled by (1-alpha)
        "syn": nc.declare_dram_parameter("syn", [H, S, S], fp16, isOutput=False),
        "out": nc.declare_dram_parameter("out", [S, D], f32, isOutput=True),
    }
    with tile.TileContext(nc) as tc:
        _emit(nc, tc, dram, c1)
    nc.compile()
    return nc


def kernel(**inputs):
    global LAST_RESULTS
    q = np.asarray(inputs["query"], np.float32)
    k = np.asarray(inputs["key"], np.float32)
    v = np.asarray(inputs["value"], np.float32)
    msk = np.asarray(inputs["mask"], np.int32)
    ws = {nm: np.asarray(inputs["W" + nm], np.float32) for nm in "qkvo"}
    bs = {nm: np.asarray(inputs["b" + nm], np.float32) for nm in "qkvo"}
    alpha = float(1.0 / (1.0 + math.exp(-float(np.asarray(inputs["alpha_param"]).ravel()[0]))))
    c1 = alpha / math.sqrt(HD)
    c2 = 1.0 - alpha
    syn_h = np.ascontiguousarray(
        (np.asarray(inputs["syn_scores"])[:, :S, :S] * c2).transpose(0, 2, 1)
    ).astype(np.float16)
    boeff = (bs["v"].astype(np.float64) @ ws["o"].astype(np.float64)
             + bs["o"]).astype(np.float32)
    w16 = {nm: ws[nm].astype(np.float16) for nm in "qkvo"}

    key_ = (round(c1, 12), round(c2, 12), "v6", RECIP_ENGINE, SYNM_VIA_ACCUM_DMA, WARMUP_MMS)
    if key_ not in _CACHE:
        _CACHE[key_] = _build(c1)
    nc = _CACHE[key_]

    in_maps = []
    for b in range(B):
        in_maps.append({
            "xq": q[b].astype(np.float16),
            "xk": k[b].astype(np.float16),
            "xv": v[b].astype(np.float16),
            "msk": msk[b].astype(np.float16),
            "wq": w16["q"], "wk": w16["k"], "wv": w16["v"], "wo": w16["o"],
            "bq": bs["q"], "bk": bs["k"], "boeff": boeff,
            "syn": syn_h,
        })

    kwargs = {}
    if TRACE:
        kwargs["trace"] = True
        if TRACE_TMPDIR:
            kwargs["tmpdir"] = TRACE_TMPDIR
    res = run_bass_kernel_spmd(nc, in_maps, core_ids=list(range(N_CORES)), **kwargs)
    LAST_RESULTS = res
    return np.stack([res.results[b]["out"] for b in range(B)], axis=0)


# revision 13
# speedup vs baseline: 1.1434x; 1.1434x over previous
"""Multi-head attention with random-synthesizer blend + mask, on 8 Trainium2
NeuronCores.

Sharding: data-parallel over batch (B=8 -> one batch element per core).

Per-core algorithm (S=1024, D=1024, H=16, HD=64), all layouts [partition, free]:
  - q_T = c1*(Wq^T x^T + bq) in [d_out, s] layout; k_T likewise (scale 1);
    v in natural [s, d_out] layout interleaved with all-ones 64-column blocks
    (used to compute softmax sums for free in the AV matmul).
  - x^T and mask^T tiles come straight from HBM via hardware xbar transpose
    reads (inputs are pre-cast to fp16 on the host, making the 2-byte
    transpose path legal with no DRAM bounce). mask^T is converted in-place
    to lmask = (mask-1)*30000 so the mask folds ADDITIVELY into the
    synthesizer scores: masked positions exp() to exactly 0.
  - Attention runs per head-PAIR so the two K=64 score matmuls of the pair
    land in different PE row-groups (partitions 0:64 vs 64:128) and execute
    concurrently. Per (pair, k-chunk): synm = synT_scaled + lmaskT (DVE),
    scores psum[128,1024] = kT^T qT += identity @ synm, p = exp(psum) in one
    [128,1024] ACT call, out/sums accumulated = [v|ones]^T p. The AV matmuls
    are emitted one k-chunk behind the score matmuls so the PE never waits
    on the exp (software pipelining; the PE queue is FIFO).
  - Normalization is pipelined: each pair's bundle is split into three
    stages (evac+swap / reciprocal / multiply) emitted one k-chunk apart
    inside the NEXT pair's loop, so every stage's dependencies are complete
    before the engine FIFOs reach it (no head-of-line blocking). The
    reciprocal runs as an elementwise divide on the otherwise-idle GPSIMD
    engine (the DVE reciprocal costs 6.5us per pair and would make DVE the
    critical engine; ACT Ln/Exp would thrash activation table sets).
  - A short burst of dummy matmuls at the start warms the PE HAM clock gate
    (cold PE runs at 1.2 GHz vs 2.4 GHz warm) while the first DMAs land.

fp16 is used for matmul operands (fp32 matmuls stream 4x slower); PSUM
accumulation stays fp32.

Host-side prep is limited to dtype casts/layout of parameters and inputs
(fp16 casts of x/mask/weights, transpose+(1-alpha)-scale of the synthesizer
scores), the sigmoid of the scalar alpha parameter, and folding the
zero-cost bias identity bo' = bv @ Wo + bo (exact: softmax weights sum to
1). alpha is folded into compiled constants; the program is rebuilt if
alpha changes.
"""

import math
import sys

sys.path.insert(0, "/opt/trn_rl_repo")

import numpy as np

import concourse.tile as tile
import concourse.mybir as mybir
from concourse import bacc
from concourse.bass_utils import run_bass_kernel_spmd
from concourse.masks import make_identity

B, S, D, H = 8, 1024, 1024, 16
HD = D // H  # 64
N_CORES = 8
P = 128
SC = S // P  # 8
DC = D // P  # 8
NQ = 512
QC = S // NQ  # 2

f32 = mybir.dt.float32
fp16 = mybir.dt.float16
i32 = mybir.dt.int32
AF = mybir.ActivationFunctionType
OP = mybir.AluOpType

# knobs
RECIP_ENGINE = "dve"  # gpsimd divide is rejected by walrus
SYNM_VIA_ACCUM_DMA = False  # build synm = lmask + syn via SWDGE accumulate-DMA
WARMUP_MMS = 32

# test harness knobs (the grading entry point `kernel` leaves these alone)
TRACE = False
TRACE_TMPDIR = None
LAST_RESULTS = None

_CACHE = {}


def _emit(nc, tc, dram, c1):
    xin = {"q": dram["xq"], "k": dram["xk"], "v": dram["xv"]}
    w_d = {"q": dram["wq"], "k": dram["wk"], "v": dram["wv"], "o": dram["wo"]}
    msk_d, syn_d, out_d = dram["msk"], dram["syn"], dram["out"]

    with tc.tile_pool(name="pers", bufs=1) as pers:
        # ---- constants ---------------------------------------------------
        ident = pers.tile([P, P], fp16, tag="ident")
        make_identity(nc, ident[:])
        ones_h = pers.tile([1, P], fp16, tag="ones_h")
        nc.vector.memset(ones_h[:], 1.0)
        ones_v = pers.tile([P, S], fp16, tag="ones_v")
        nc.vector.memset(ones_v[:], 1.0)

        # bo' = bv @ Wo + bo, prepared by the host into dram["boeff"]
        bo_sb = pers.tile([1, D], fp16, tag="bo_sb")

        # ---- persistent activations --------------------------------------
        qT = [pers.tile([P, S], fp16, tag=f"qT{i}", name=f"qT{i}") for i in range(DC)]
        kT = [pers.tile([P, S], fp16, tag=f"kT{i}", name=f"kT{i}") for i in range(DC)]
        # lmaskT[kc] = (maskT - 1) * 30000  (0 where visible, -30000 masked)
        lmaskT = [pers.tile([P, S], fp16, tag=f"lm{i}", name=f"lm{i}")
                  for i in range(SC)]

        def load_w_chunks(nm, wpool, wbufs=2):
            tiles = []
            for ci in range(DC):
                t = wpool.tile([P, D], fp16, tag=f"w{ci}", bufs=wbufs, name=f"w{nm}{ci}")
                nc.sync.dma_start(out=t[:], in_=w_d[nm][ci * P:(ci + 1) * P, :])
                tiles.append(t)
            return tiles

        # ================= phase 1: projections ==========================
        with (
            tc.tile_pool(name="prolog", bufs=1) as pro,
            tc.tile_pool(name="ps1", bufs=1, space="PSUM") as ps1,
        ):
            # PE warmup: dependency-free matmuls so HAM un-throttles
            # (1.2 -> 2.4 GHz) while the first DMAs land.
            wscr = pro.tile([P, NQ], fp16, tag="wscr")
            nc.vector.memset(wscr[:], 0.25)
            wup = ps1.tile([P, S], f32, tag="mmp", bufs=4, name="wup")
            for _ in range(WARMUP_MMS):
                nc.tensor.matmul(wup[:, 0:NQ], ident[:], wscr[:],
                                 start=True, stop=True)

            def transpose_in(x_d, dst_tiles):
                for di in range(DC):
                    nc.sync.dma_start_transpose(
                        out=dst_tiles[di][:], in_=x_d[:, di * P:(di + 1) * P]
                    )

            # q_T / k_T: [d_out, s]; di-outer so the stationary operand
            # (weight chunk) is shared by the sq pair
            bqk_sb = {}
            for nm, dst, scale in (("q", qT, c1), ("k", kT, 1.0)):
                xT = [pro.tile([P, S], fp16, tag=f"xT{i}", bufs=2, name=f"xT{nm}{i}")
                      for i in range(DC)]
                transpose_in(xin[nm], xT)
                wt = load_w_chunks(nm, pro)
                t = pers.tile([P, DC], f32, tag=f"b{nm}", name=f"b{nm}")
                nc.sync.dma_start(
                    out=t[:], in_=dram["b" + nm].rearrange("(c p) -> p c", p=P))
                bqk_sb[nm] = t
                if nm == "q" and c1 != 1.0:
                    nc.vector.tensor_scalar(
                        out=t[:], in0=t[:], scalar1=float(c1),
                        scalar2=None, op0=OP.mult,
                    )
                for do in range(DC):
                    ps = ps1.tile([P, S], f32, tag="mmp", bufs=4, name="psp")
                    for di in range(DC):
                        for sq in range(QC):
                            nc.tensor.matmul(
                                ps[:, sq * NQ:(sq + 1) * NQ],
                                wt[di][:, do * P:(do + 1) * P],
                                xT[di][:, sq * NQ:(sq + 1) * NQ],
                                start=(di == 0),
                                stop=(di == DC - 1),
                            )
                    nc.scalar.activation(
                        out=dst[do][:], in_=ps[:],
                        func=AF.Identity, bias=bqk_sb[nm][:, do:do + 1],
                        scale=float(scale),
                    )

            b0 = pro.tile([1, D], f32, tag="braw")
            nc.sync.dma_start(out=b0[:], in_=dram["boeff"][None, :])
            nc.vector.tensor_copy(out=bo_sb[:], in_=b0[:])

            # v natural [s, d_out] into interleaved [v|ones] blocks (fp16);
            # dq-inner so the stationary operand (xT chunk) is shared
            v_sb = [pers.tile([P, H * P], fp16, tag=f"v{i}", name=f"v{i}")
                    for i in range(SC)]
            xT = [pro.tile([P, S], fp16, tag=f"xT{i}", bufs=2, name=f"xTv{i}")
                  for i in range(DC)]
            transpose_in(xin["v"], xT)
            wt = load_w_chunks("v", pro)
            for sc in range(SC):
                nc.vector.memset(v_sb[sc][:], 1.0)
            for sc in range(SC):
                ps = ps1.tile([P, S], f32, tag="mmp", bufs=4, name="psv")
                for di in range(DC):
                    for dq in range(QC):
                        nc.tensor.matmul(
                            ps[:, dq * NQ:(dq + 1) * NQ],
                            xT[di][:, sc * P:(sc + 1) * P],
                            wt[di][:, dq * NQ:(dq + 1) * NQ],
                            start=(di == 0),
                            stop=(di == DC - 1),
                        )
                # heads h = j here; v block of h at col h*128 + 64*(h&1).
                # Two strided copies instead of 16 tiny ones.
                pv = ps[:].rearrange("p (b c) -> p b c", c=2 * HD)
                dv = v_sb[sc][:].rearrange("p (b c) -> p b c", c=2 * P)
                nc.scalar.copy(out=dv[:, :, 0:HD], in_=pv[:, :, 0:HD])
                nc.scalar.copy(out=dv[:, :, 3 * HD:4 * HD],
                               in_=pv[:, :, HD:2 * HD])

            # mask^T + lmask transform (needed only at attention time)
            for kb in range(SC):
                nc.sync.dma_start_transpose(
                    out=lmaskT[kb][:], in_=msk_d[:, kb * P:(kb + 1) * P]
                )
                nc.vector.tensor_scalar(
                    out=lmaskT[kb][:], in0=lmaskT[kb][:],
                    scalar1=1.0, scalar2=30000.0,
                    op0=OP.subtract, op1=OP.mult,
                )


        # ================= phase 2: attention ============================
        otnp_cm = tc.tile_pool(name="otnp", bufs=1)
        otnp = otnp_cm.__enter__()
        wop_cm = tc.tile_pool(name="wo", bufs=1)
        wop = wop_cm.__enter__()
        otn = [otnp.tile([P, S], fp16, tag=f"otn{i}", name=f"otn{i}")
               for i in range(DC)]
        otn_raw = [otnp.tile([P, S], fp16, tag=f"otr{i}", name=f"otr{i}")
                   for i in range(DC)]
        sums_sb = [otnp.tile([P, S], fp16, tag=f"sus{i}", name=f"sus{i}")
                   for i in range(DC)]
        # prefetch Wo (DMA has slack during attention)
        wt_o = load_w_chunks("o", wop, wbufs=1)
        with (
            tc.tile_pool(name="attn", bufs=1) as ap,
            tc.tile_pool(name="ps2", bufs=1, space="PSUM") as ps2,
        ):
            def make_norm_stages(hp, pav_box):
                h0, h1 = 2 * hp, 2 * hp + 1

                def s1_evac_swap():
                    pav = pav_box["pav"]
                    for i, hh in enumerate((h0, h1)):
                        olo, slo = HD * (hh % 2), HD * (1 - hh % 2)
                        nc.scalar.copy(
                            out=otn_raw[hp][olo:olo + HD, :],
                            in_=pav[i][olo:olo + HD, :],
                        )
                        nc.vector.tensor_copy(
                            out=sums_sb[hp][slo:slo + HD, :],
                            in_=pav[i][slo:slo + HD, :],
                        )
                    rt = ap.tile([P, S], fp16, tag="rtm", bufs=2, name=f"rt{hp}")
                    nc.sync.dma_start(out=rt[0:HD, :], in_=sums_sb[hp][HD:P, :])
                    nc.sync.dma_start(out=rt[HD:P, :], in_=sums_sb[hp][0:HD, :])
                    st["rt"] = rt

                def s2_recip():
                    rec = ap.tile([P, S], fp16, tag="rec", bufs=2, name=f"rc{hp}")
                    with nc.allow_low_precision(reason="softmax denominators; fp16 rel err 5e-4 is fine"):
                        nc.vector.reciprocal(out=rec[:], in_=st["rt"][:])
                    st["rec"] = rec

                def s3_mult():
                    nc.vector.tensor_tensor(
                        out=otn[hp][:], in0=otn_raw[hp][:], in1=st["rec"][:],
                        op=OP.mult,
                    )

                st = {}
                return [None, s1_evac_swap, s2_recip, s3_mult]

            def make_av_emitter(hp, pav_box, p0, p1, kc):
                def emit():
                    if "pav" not in pav_box:
                        pav_box["pav"] = [
                            ps2.tile([P, S], f32, tag=f"av{i}", bufs=1,
                                     name=f"pav{hp}_{i}")
                            for i in range(2)
                        ]
                    pav = pav_box["pav"]
                    for i, p in enumerate((p0, p1)):
                        hh = 2 * hp + i
                        for sq in range(QC):
                            sl = slice(sq * NQ, (sq + 1) * NQ)
                            nc.tensor.matmul(
                                pav[i][:, sl],
                                v_sb[kc][:, hh * P:(hh + 1) * P], p[:, sl],
                                start=(kc == 0), stop=(kc == SC - 1),
                            )
                return emit

            pending = []
            prev_av = None
            for hp in range(H // 2):
                h0, h1 = 2 * hp, 2 * hp + 1
                pav_box = {}
                for kc in range(SC):
                    syns = []
                    for i, hh in enumerate((h0, h1)):
                        synm = ap.tile([P, S], fp16, tag="synm", bufs=6,
                                       name=f"sm{hh}_{kc}")
                        if SYNM_VIA_ACCUM_DMA:
                            # prefill with lmask, then stream syn from HBM
                            # with an inline add (SWDGE accumulate-DMA): the
                            # add costs no DVE time at all
                            nc.vector.tensor_copy(out=synm[:], in_=lmaskT[kc][:])
                            nc.gpsimd.dma_start(
                                out=synm[:],
                                in_=syn_d[hh, kc * P:(kc + 1) * P, :],
                                accum_op=OP.add,
                            )
                        else:
                            syn_t = ap.tile([P, S], fp16, tag="synT", bufs=6,
                                            name=f"sy{hh}_{kc}")
                            nc.sync.dma_start(
                                out=syn_t[:],
                                in_=syn_d[hh, kc * P:(kc + 1) * P, :],
                            )
                            nc.vector.tensor_tensor(
                                out=synm[:], in0=syn_t[:], in1=lmaskT[kc][:],
                                op=OP.add,
                            )
                        syns.append(synm)
                    pss = [ps2.tile([P, S], f32, tag="mm", bufs=2,
                                    name=f"pss{i}") for i in range(2)]
                    # score matmuls, pair-interleaved: h0 uses PE rows 0:64,
                    # h1 rows 64:128 -> adjacent matmuls run concurrently
                    for sq in range(QC):
                        sl = slice(sq * NQ, (sq + 1) * NQ)
                        for i in range(2):
                            nc.tensor.matmul(
                                pss[i][:, sl],
                                kT[hp][i * HD:(i + 1) * HD, kc * P:(kc + 1) * P],
                                qT[hp][i * HD:(i + 1) * HD, sl],
                                start=True, stop=False,
                            )
                    for sq in range(QC):
                        sl = slice(sq * NQ, (sq + 1) * NQ)
                        for i in range(2):
                            nc.tensor.matmul(
                                pss[i][:, sl], ident[:], syns[i][:, sl],
                                start=False, stop=True,
                            )
                    # one stage of the pair-before-last's normalization, then
                    # the AV matmuls lagging one k-chunk behind (the lag runs
                    # across pair boundaries, so the PE stream never waits on
                    # a just-issued exp)
                    if pending:
                        s = pending.pop(0)
                        if s is not None:
                            s()
                    if prev_av is not None:
                        prev_av()
                    ps_ = []
                    for i in range(2):
                        p = ap.tile([P, S], fp16, tag="p", bufs=5, name=f"p{i}")
                        nc.scalar.activation(out=p[:], in_=pss[i][:], func=AF.Exp)
                        ps_.append(p)
                    prev_av = make_av_emitter(hp, pav_box, ps_[0], ps_[1], kc)
                pending = make_norm_stages(hp, pav_box)

            prev_av()
            while pending:
                s = pending.pop(0)
                if s is not None:
                    s()

        # ================= phase 3: output projection ====================
        with tc.tile_pool(name="ps3", bufs=1, space="PSUM") as ps3:
            for sc in range(SC):
                ps = ps3.tile([P, S], f32, tag="mmo", bufs=3, name="pso")
                for ci in range(DC):
                    for dq in range(QC):
                        sl = slice(dq * NQ, (dq + 1) * NQ)
                        nc.tensor.matmul(
                            ps[:, sl],
                            otn[ci][:, sc * P:(sc + 1) * P],
                            wt_o[ci][:, sl],
                            start=(ci == 0), stop=False,
                        )
                for dq in range(QC):
                    sl = slice(dq * NQ, (dq + 1) * NQ)
                    nc.tensor.matmul(
                        ps[:, sl], ones_h[:, :P], bo_sb[:, sl],
                        start=False, stop=True,
                    )
                osb = wop.tile([P, S], f32, tag="osb", bufs=3, name="osb")
                nc.scalar.copy(out=osb[:], in_=ps[:])
                nc.sync.dma_start(
                    out=out_d[sc * P:(sc + 1) * P, :], in_=osb[:],
                )
        wop_cm.__exit__(None, None, None)
        otnp_cm.__exit__(None, None, None)


def _build(c1):
    nc = bacc.Bacc("TRN2", debug=False)
    dram = {
        "xq": nc.declare_dram_parameter("xq", [S, D], fp16, isOutput=False),
        "xk": nc.declare_dram_parameter("xk", [S, D], fp16, isOutput=False),
        "xv": nc.declare_dram_parameter("xv", [S, D], fp16, isOutput=False),
        "msk": nc.declare_dram_parameter("msk", [S, S], fp16, isOutput=False),
        "wq": nc.declare_dram_parameter("wq", [D, D], fp16, isOutput=False),
        "wk": nc.declare_dram_parameter("wk", [D, D], fp16, isOutput=False),
        "wv": nc.declare_dram_parameter("wv", [D, D], fp16, isOutput=False),
        "wo": nc.declare_dram_parameter("wo", [D, D], fp16, isOutput=False),
        "bq": nc.declare_dram_parameter("bq", [D], f32, isOutput=False),
        "bk": nc.declare_dram_parameter("bk", [D], f32, isOutput=False),
        "boeff": nc.declare_dram_parameter("boeff", [D], f32, isOutput=False),
        # pre-transposed [H, S_k, S_q] and pre-scaled by (1-alpha)
        "syn": nc.declare_dram_parameter("syn", [H, S, S], fp16, isOutput=False),
        "out": nc.declare_dram_parameter("out", [S, D], f32, isOutput=True),
    }
    with tile.TileContext(nc) as tc:
        _emit(nc, tc, dram, c1)
    nc.compile()
    return nc


def kernel(**inputs):
    global LAST_RESULTS
    q = np.asarray(inputs["query"], np.float32)
    k = np.asarray(inputs["key"], np.float32)
    v = np.asarray(inputs["value"], np.float32)
    msk = np.asarray(inputs["mask"], np.int32)
    ws = {nm: np.asarray(inputs["W" + nm], np.float32) for nm in "qkvo"}
    bs = {nm: np.asarray(inputs["b" + nm], np.float32) for nm in "qkvo"}
    alpha = float(1.0 / (1.0 + math.exp(-float(np.asarray(inputs["alpha_param"]).ravel()[0]))))
    c1 = alpha / math.sqrt(HD)
    c2 = 1.0 - alpha
    syn_h = np.ascontiguousarray(
        (np.asarray(inputs["syn_scores"])[:, :S, :S] * c2).transpose(0, 2, 1)
    ).astype(np.float16)
    boeff = (bs["v"].astype(np.float64) @ ws["o"].astype(np.float64)
             + bs["o"]).astype(np.float32)
    w16 = {nm: ws[nm].astype(np.float16) for nm in "qkvo"}

    key_ = (round(c1, 12), round(c2, 12), "v7", RECIP_ENGINE, SYNM_VIA_ACCUM_DMA, WARMUP_MMS)
    if key_ not in _CACHE:
        _CACHE[key_] = _build(c1)
    nc = _CACHE[key_]

    in_maps = []
    for b in range(B):
        in_maps.append({
            "xq": q[b].astype(np.float16),
            "xk": k[b].astype(np.float16),
            "xv": v[b].astype(np.float16),
            "msk": msk[b].astype(np.float16),
            "wq": w16["q"], "wk": w16["k"], "wv": w16["v"], "wo": w16["o"],
            "bq": bs["q"], "bk": bs["k"], "boeff": boeff,
            "syn": syn_h,
        })

    kwargs = {}
    if TRACE:
        kwargs["trace"] = True
        if TRACE_TMPDIR:
            kwargs["tmpdir"] = TRACE_TMPDIR
    res = run_bass_kernel_spmd(nc, in_maps, core_ids=list(range(N_CORES)), **kwargs)
    LAST_RESULTS = res
    return np.stack([res.results[b]["out"] for b in range(B)], axis=0)
